# revision 13
# baseline (speedup 1.0000x reference)
"""Trainium2 Bass kernel for binarized 3x3 conv + batch-norm (BinConv2d).

Reference computation:
    xb = sign(x); wb = sign(weight)
    y  = conv2d(xb, wb, stride 1, pad 1)        # NCHW / OIHW
    out = batchnorm(y, batch stats over (N,H,W), affine gamma/beta)

Strategy: data-parallel over batch (64 images -> 8 images on each of the 8
NeuronCores). Conv is expressed as 9 shifted matmuls (one per 3x3 tap) with
Cin=128 on the SBUF partition dim, accumulating into PSUM. Sign products are
exactly representable in bf16 and PSUM accumulates in fp32, so the conv is
exact. Conv outputs are integers |y| <= 1152, stored in SBUF as fp16 (exact).
Per-channel sum / sum-of-squares are reduced on-core, AllReduced across the
8 cores, and the affine (scale, bias) is applied on-device before the f32
output DMA.
"""
import numpy as np

import concourse.bacc as bacc
import concourse.tile as tile
import concourse.mybir as mybir
import concourse.bass_utils as bass_utils

F32 = mybir.dt.float32
F16 = mybir.dt.float16
BF16 = mybir.dt.bfloat16
AF = mybir.ActivationFunctionType
ALU = mybir.AluOpType

N_CORES = 8
N_FULL = 64          # total batch
NIMG = N_FULL // N_CORES   # images per core
C = 128              # channels (in == out)
H = W = 56
HP = WP = H + 2      # padded spatial
NT = 7               # row tiles per image
RT = H // NT         # rows per tile (8)
IMG = H * W          # 3136
COUNT = N_FULL * IMG  # global reduction count per channel
EPS = 1e-5

TRACE = False        # test.py may flip this to get an NTFF profile

_CACHE = {}


def _build(use_collective=True, nimg=NIMG):
    nc = bacc.Bacc("TRN2", target_bir_lowering=False, debug=False,
                   num_devices=N_CORES)
    x = nc.dram_tensor("x", [NIMG, C, H, W], F32, kind="ExternalInput").ap()
    wt = nc.dram_tensor("wt", [C, 9, C], F32, kind="ExternalInput").ap()
    gb = nc.dram_tensor("gb", [C, 2], F32, kind="ExternalInput").ap()
    out = nc.dram_tensor("out", [NIMG, C, H, W], F32, kind="ExternalOutput").ap()

    with tile.TileContext(nc) as tc:
        with tc.tile_pool(name="const", bufs=1) as pc, \
             tc.tile_pool(name="xstage", bufs=3) as pxs, \
             tc.tile_pool(name="xpad", bufs=3) as pxp, \
             tc.tile_pool(name="ostage", bufs=3) as pos, \
             tc.tile_pool(name="psum", bufs=8, space="PSUM") as pp, \
             tc.tile_pool(name="dram", bufs=1, space="DRAM") as pd:

            # ---- weights: load f32 [ci, tap, co], sign -> bf16 ----
            wstage = pc.tile([C, 9, C], F32)
            nc.sync.dma_start(out=wstage[:], in_=wt[:])
            wb = pc.tile([C, 9, C], BF16)
            nc.scalar.activation(out=wb[:], in_=wstage[:], func=AF.Sign)

            gbt = pc.tile([C, 2], F32)
            nc.sync.dma_start(out=gbt[:], in_=gb[:])

            # ---- persistent buffers ----
            y16 = pc.tile([C, NIMG, H, W], F16)      # conv result (exact ints)
            bnbuf = pc.tile([C, nimg * NT, 6], F32)  # bn_stats per tile
            stats = pc.tile([C, 2], F32)
            gstats = pc.tile([C, 2], F32)

            # ---- phase 1: conv + local stats, per image ----
            for n in range(nimg):
                xs = pxs.tile([C, H, W], F32)
                nc.sync.dma_start(out=xs[:], in_=x[n])

                xp = pxp.tile([C, HP, WP], BF16)
                # zero the 1-pixel border, sign-fill the interior
                nc.gpsimd.memset(xp[:, 0, :], 0.0)
                nc.gpsimd.memset(xp[:, HP - 1, :], 0.0)
                nc.gpsimd.memset(xp[:, 1:HP - 1, 0], 0.0)
                nc.gpsimd.memset(xp[:, 1:HP - 1, WP - 1], 0.0)
                nc.scalar.activation(out=xp[:, 1:HP - 1, 1:WP - 1], in_=xs[:],
                                     func=AF.Sign)

                psums = [pp.tile([C, RT, W], F32, tag="ps", name="ps")
                         for _ in range(NT)]
                for it in range(9):
                    dh, dw = it // 3 - 1, it % 3 - 1
                    for t in range(NT):
                        h0 = t * RT
                        rhs = xp[:, h0 + 1 + dh:h0 + 1 + dh + RT,
                                 1 + dw:1 + dw + W]
                        nc.tensor.matmul(out=psums[t][:], lhsT=wb[:, it, :],
                                         rhs=rhs, start=(it == 0),
                                         stop=(it == 8))

                for t in range(NT):
                    idx = n * NT + t
                    # ACT: PSUM -> fp16 copy (conv ints, exact in fp16)
                    nc.scalar.copy(out=y16[:, n, t * RT:(t + 1) * RT, :],
                                   in_=psums[t][:])
                    # DVE: count/mean/M2 in one pass straight from PSUM
                    nc.vector.bn_stats(
                        out=bnbuf[:, idx, :],
                        in_=psums[t][:].rearrange("p a b -> p (a b)"))

            # ---- phase 2: global stats via AllReduce ----
            # aggregate per-tile stats -> per-core [mean, var], then convert
            # to [sum, sumsq] which AllReduce can combine
            mv = pc.tile([C, 2], F32)
            nc.vector.bn_aggr(out=mv[:],
                              in_=bnbuf[:].rearrange("p a b -> p (a b)"))
            cnt = float(nimg * IMG)
            msq_t = pc.tile([C, 1], F32)
            nc.vector.tensor_mul(msq_t[:], mv[:, 0:1], mv[:, 0:1])
            nc.vector.tensor_scalar_mul(stats[:, 0:1], mv[:, 0:1], cnt)
            nc.vector.tensor_add(stats[:, 1:2], mv[:, 1:2], msq_t[:])
            nc.vector.tensor_scalar_mul(stats[:, 1:2], stats[:, 1:2], cnt)
            if use_collective:
                bin_ = pd.tile([C, 2], F32)
                bout = pd.tile([C, 2], F32)
                nc.sync.dma_start(out=bin_[:], in_=stats[:])
                nc.gpsimd.collective_compute(
                    "AllReduce", ALU.add,
                    replica_groups=[list(range(N_CORES))],
                    ins=[bin_.opt()], outs=[bout.opt()])
                nc.sync.dma_start(out=gstats[:], in_=bout[:])
            else:
                # local stats only (debug): scale by N_CORES to keep the
                # normalization magnitude roughly right
                nc.vector.tensor_scalar_mul(gstats[:], stats[:],
                                            float(N_CORES))

            # scale = gamma / sqrt(var + eps); bias = beta - mean * scale
            mean_t = pc.tile([C, 1], F32)
            e2_t = pc.tile([C, 1], F32)
            var_t = pc.tile([C, 1], F32)
            std_t = pc.tile([C, 1], F32)
            inv_t = pc.tile([C, 1], F32)
            scale_t = pc.tile([C, 1], F32)
            bias_t = pc.tile([C, 1], F32)
            tmp_t = pc.tile([C, 1], F32)
            inv_count = 1.0 / COUNT
            nc.vector.tensor_scalar_mul(mean_t[:], gstats[:, 0:1], inv_count)
            nc.vector.tensor_scalar_mul(e2_t[:], gstats[:, 1:2], inv_count)
            nc.vector.tensor_mul(tmp_t[:], mean_t[:], mean_t[:])
            nc.vector.tensor_sub(var_t[:], e2_t[:], tmp_t[:])
            nc.vector.tensor_scalar_add(var_t[:], var_t[:], EPS)
            nc.scalar.activation(out=std_t[:], in_=var_t[:], func=AF.Sqrt)
            nc.vector.reciprocal(inv_t[:], std_t[:])
            nc.vector.tensor_mul(scale_t[:], gbt[:, 0:1], inv_t[:])
            nc.vector.tensor_mul(tmp_t[:], mean_t[:], scale_t[:])
            nc.vector.tensor_sub(bias_t[:], gbt[:, 1:2], tmp_t[:])

            # ---- phase 3: affine + store (split across ACT and DVE) ----
            for n in range(nimg):
                ot = pos.tile([C, H, W], F32)
                nc.vector.tensor_scalar(
                    ot[:], y16[:, n], scale_t[:, 0:1], bias_t[:, 0:1],
                    ALU.mult, ALU.add)
                nc.sync.dma_start(out=out[n], in_=ot[:])

    nc.compile()
    return nc


def kernel(x, weight, gamma, beta):
    x = np.asarray(x, dtype=np.float32)
    weight = np.asarray(weight, dtype=np.float32)
    gamma = np.asarray(gamma, dtype=np.float32)
    beta = np.asarray(beta, dtype=np.float32)

    if "nc" not in _CACHE:
        _CACHE["nc"] = _build()
    nc = _CACHE["nc"]

    # wt[ci, kh*3+kw, co] = weight[co, ci, kh, kw]
    wt = np.ascontiguousarray(weight.transpose(1, 2, 3, 0)).reshape(C, 9, C)
    gb = np.ascontiguousarray(np.stack([gamma, beta], axis=1))

    in_maps = []
    for i in range(N_CORES):
        in_maps.append({
            "x": np.ascontiguousarray(x[i * NIMG:(i + 1) * NIMG]),
            "wt": wt,
            "gb": gb,
        })

    res = bass_utils.run_bass_kernel_spmd(
        nc, in_maps, core_ids=list(range(N_CORES)), trace=TRACE)
    _CACHE["last_result"] = res

    out = np.empty((N_FULL, C, H, W), dtype=np.float32)
    for i in range(N_CORES):
        out[i * NIMG:(i + 1) * NIMG] = res.results[i]["out"]
    return out


# revision 17
# speedup vs baseline: 1.1728x; 1.1728x over previous
"""Trainium2 Bass kernel for binarized 3x3 conv + batch-norm (BinConv2d).

Reference computation:
    xb = sign(x); wb = sign(weight)
    y  = conv2d(xb, wb, stride 1, pad 1)        # NCHW / OIHW
    out = batchnorm(y, batch stats over (N,H,W), affine gamma/beta)

Strategy: data-parallel over batch (64 images -> 8 images per NeuronCore).
The conv runs as shifted matmuls with Cin=128 on the SBUF partition dim,
accumulating in PSUM. Signs are cast to fp8 (e4m3, +/-1 exact) and the 3x3
taps are processed as 4 DoubleRow pairs + 1 single matmul per output tile
(~1.8x TensorE throughput vs bf16). Matmul tiles span 8 rows x 58 cols of
the zero-padded image so every tap's moving operand is one contiguous
464-element run; the two junk columns per row are skipped downstream.
Conv outputs are integers |y| <= 1152: exact in fp32 PSUM and in the fp16
SBUF copy. Channel stats come from DVE bn_stats/bn_aggr, are AllReduced
across the 8 cores, and the affine is applied on-device before the f32
output DMA.
"""
import numpy as np

import concourse.bacc as bacc
import concourse.tile as tile
import concourse.mybir as mybir
import concourse.bass_utils as bass_utils
from concourse.bass_types import AP

F32 = mybir.dt.float32
F16 = mybir.dt.float16
F8 = mybir.dt.float8e4
AF = mybir.ActivationFunctionType
ALU = mybir.AluOpType
DR = mybir.MatmulPerfMode.DoubleRow

N_CORES = 8
N_FULL = 64            # total batch
NIMG = N_FULL // N_CORES   # images per core
C = 128                # channels (in == out)
H = W = 56
WP = W + 2             # padded width (58)
HPHYS = H + 4          # physical rows: guard + pad + 56 + pad + guard
PSTRIDE = HPHYS * WP   # per-partition elements of one image tile
NT = 7                 # row tiles per image
RT = H // NT           # rows per tile (8)
TW = RT * WP           # moving free size per tile (464)
IMG = H * W            # 3136
COUNT = N_FULL * IMG   # global reduction count per channel
EPS = 1e-5

TRACE = False          # test.py may flip this to get an NTFF profile

_CACHE = {}


def _build(use_collective=True, nimg=NIMG):
    nc = bacc.Bacc("TRN2", target_bir_lowering=False, debug=False,
                   num_devices=N_CORES)
    x = nc.dram_tensor("x", [NIMG, C, H, W], F32, kind="ExternalInput").ap()
    wt = nc.dram_tensor("wt", [C, 9, C], F32, kind="ExternalInput").ap()
    gb = nc.dram_tensor("gb", [C, 2], F32, kind="ExternalInput").ap()
    out = nc.dram_tensor("out", [NIMG, C, H, W], F32, kind="ExternalOutput").ap()

    with tile.TileContext(nc) as tc:
        with tc.tile_pool(name="const", bufs=1) as pc, \
             tc.tile_pool(name="xstage", bufs=3) as pxs, \
             tc.tile_pool(name="xpad", bufs=3) as pxp, \
             tc.tile_pool(name="ostage", bufs=3) as pos, \
             tc.tile_pool(name="psum", bufs=8, space="PSUM") as pp, \
             tc.tile_pool(name="dram", bufs=1, space="DRAM") as pd:

            # ---- weights: load f32 [ci, tap, co], sign -> fp8 ----
            wstage = pc.tile([C, 9, C], F32)
            nc.sync.dma_start(out=wstage[:], in_=wt[:])
            wb = pc.tile([C, 9, C], F8)
            nc.scalar.activation(out=wb[:], in_=wstage[:], func=AF.Sign)

            gbt = pc.tile([C, 2], F32)
            nc.sync.dma_start(out=gbt[:], in_=gb[:])

            # ---- persistent buffers ----
            y16 = pc.tile([C, NIMG, H, W], F16)       # conv ints (exact)
            bnbuf = pc.tile([C, nimg * NT, 6], F32)
            stats = pc.tile([C, 2], F32)
            gstats = pc.tile([C, 2], F32)

            # ---- phase 1: conv + local stats, per image ----
            for n in range(nimg):
                xs = pxs.tile([C, H, W], F32)
                nc.sync.dma_start(out=xs[:], in_=x[n])

                # physical rows: 0 guard, 1 top pad, 2..57 image, 58 bottom
                # pad, 59 guard. Guards keep the deliberate 2-junk-column
                # overreads of the 58-wide matmul tiles inside the tile.
                xp = pxp.tile([C, HPHYS, WP], F8)
                nc.gpsimd.memset(xp[:, 0:2, :], 0.0)
                nc.gpsimd.memset(xp[:, HPHYS - 2:HPHYS, :], 0.0)
                nc.gpsimd.memset(xp[:, 2:HPHYS - 2, 0], 0.0)
                nc.gpsimd.memset(xp[:, 2:HPHYS - 2, WP - 1], 0.0)
                nc.scalar.activation(out=xp[:, 2:HPHYS - 2, 1:WP - 1],
                                     in_=xs[:], func=AF.Sign)

                psums = [pp.tile([C, TW], F32, tag="ps", name="ps")
                         for _ in range(NT)]

                def tap_off(h0, it):
                    dh, dw = it // 3 - 1, it % 3 - 1
                    return (h0 + 2 + dh) * WP + dw

                for t in range(NT):
                    h0 = t * RT
                    # 4 DoubleRow pairs: taps (0,1),(2,3),(4,5),(6,7)
                    for p in range(4):
                        o0 = tap_off(h0, 2 * p)
                        o1 = tap_off(h0, 2 * p + 1)
                        rhs = AP(xp.tensor, xp.offset + o0,
                                 [[PSTRIDE, C], [o1 - o0, 2], [1, TW]])
                        nc.tensor.matmul(out=psums[t][:],
                                         lhsT=wb[:, 2 * p:2 * p + 2, :],
                                         rhs=rhs, start=(p == 0), stop=False,
                                         perf_mode=DR)
                    # final single tap 8
                    o8 = tap_off(h0, 8)
                    rhs8 = AP(xp.tensor, xp.offset + o8,
                              [[PSTRIDE, C], [1, TW]])
                    nc.tensor.matmul(out=psums[t][:], lhsT=wb[:, 8, :],
                                     rhs=rhs8, start=False, stop=True)

                for t in range(NT):
                    idx = n * NT + t
                    ps3 = psums[t][:].rearrange("p (r c) -> p r c", r=RT)
                    # ACT: PSUM -> fp16 copy of the 56 valid columns
                    nc.scalar.copy(out=y16[:, n, t * RT:(t + 1) * RT, :],
                                   in_=ps3[:, :, 1:W + 1])
                    # DVE: count/mean/M2 from the contiguous fp16 copy
                    nc.vector.bn_stats(
                        out=bnbuf[:, idx, :],
                        in_=y16[:, n, t * RT:(t + 1) * RT, :].rearrange(
                            "p r c -> p (r c)"))

            # ---- phase 2: global stats via AllReduce ----
            mv = pc.tile([C, 2], F32)
            nc.vector.bn_aggr(out=mv[:],
                              in_=bnbuf[:].rearrange("p a s -> p (a s)"))
            cnt = float(nimg * IMG)
            msq_t = pc.tile([C, 1], F32)
            nc.vector.tensor_mul(msq_t[:], mv[:, 0:1], mv[:, 0:1])
            nc.vector.tensor_scalar_mul(stats[:, 0:1], mv[:, 0:1], cnt)
            nc.vector.tensor_add(stats[:, 1:2], mv[:, 1:2], msq_t[:])
            nc.vector.tensor_scalar_mul(stats[:, 1:2], stats[:, 1:2], cnt)
            if use_collective:
                bin_ = pd.tile([C, 2], F32)
                bout = pd.tile([C, 2], F32)
                nc.sync.dma_start(out=bin_[:], in_=stats[:])
                nc.gpsimd.collective_compute(
                    "AllReduce", ALU.add,
                    replica_groups=[list(range(N_CORES))],
                    ins=[bin_.opt()], outs=[bout.opt()])
                nc.sync.dma_start(out=gstats[:], in_=bout[:])
            else:
                nc.vector.tensor_scalar_mul(gstats[:], stats[:],
                                            float(N_CORES))

            # scale = gamma / sqrt(var + eps); bias = beta - mean * scale
            mean_t = pc.tile([C, 1], F32)
            e2_t = pc.tile([C, 1], F32)
            var_t = pc.tile([C, 1], F32)
            std_t = pc.tile([C, 1], F32)
            inv_t = pc.tile([C, 1], F32)
            scale_t = pc.tile([C, 1], F32)
            bias_t = pc.tile([C, 1], F32)
            tmp_t = pc.tile([C, 1], F32)
            inv_count = 1.0 / COUNT
            nc.vector.tensor_scalar_mul(mean_t[:], gstats[:, 0:1], inv_count)
            nc.vector.tensor_scalar_mul(e2_t[:], gstats[:, 1:2], inv_count)
            nc.vector.tensor_mul(tmp_t[:], mean_t[:], mean_t[:])
            nc.vector.tensor_sub(var_t[:], e2_t[:], tmp_t[:])
            nc.vector.tensor_scalar_add(var_t[:], var_t[:], EPS)
            nc.scalar.activation(out=std_t[:], in_=var_t[:], func=AF.Sqrt)
            nc.vector.reciprocal(inv_t[:], std_t[:])
            nc.vector.tensor_mul(scale_t[:], gbt[:, 0:1], inv_t[:])
            nc.vector.tensor_mul(tmp_t[:], mean_t[:], scale_t[:])
            nc.vector.tensor_sub(bias_t[:], gbt[:, 1:2], tmp_t[:])

            # ---- phase 3: affine + store, half-image chunks on ACT+DVE ----
            HH = H // 2
            for n in range(nimg):
                ot = pos.tile([C, H, W], F32)
                nc.vector.tensor_scalar(
                    ot[:, 0:HH, :], y16[:, n, 0:HH, :],
                    scale_t[:, 0:1], bias_t[:, 0:1], ALU.mult, ALU.add)
                nc.scalar.activation(
                    out=ot[:, HH:H, :], in_=y16[:, n, HH:H, :],
                    func=AF.Identity, bias=bias_t[:, 0:1],
                    scale=scale_t[:, 0:1])
                nc.sync.dma_start(out=out[n], in_=ot[:])

    nc.compile()
    return nc


def kernel(x, weight, gamma, beta):
    x = np.asarray(x, dtype=np.float32)
    weight = np.asarray(weight, dtype=np.float32)
    gamma = np.asarray(gamma, dtype=np.float32)
    beta = np.asarray(beta, dtype=np.float32)

    if "nc" not in _CACHE:
        _CACHE["nc"] = _build()
    nc = _CACHE["nc"]

    # wt[ci, kh*3+kw, co] = weight[co, ci, kh, kw]
    wt = np.ascontiguousarray(weight.transpose(1, 2, 3, 0)).reshape(C, 9, C)
    gb = np.ascontiguousarray(np.stack([gamma, beta], axis=1))

    in_maps = []
    for i in range(N_CORES):
        in_maps.append({
            "x": np.ascontiguousarray(x[i * NIMG:(i + 1) * NIMG]),
            "wt": wt,
            "gb": gb,
        })

    res = bass_utils.run_bass_kernel_spmd(
        nc, in_maps, core_ids=list(range(N_CORES)), trace=TRACE)
    _CACHE["last_result"] = res

    out = np.empty((N_FULL, C, H, W), dtype=np.float32)
    for i in range(N_CORES):
        out[i * NIMG:(i + 1) * NIMG] = res.results[i]["out"]
    return out


# revision 20
# speedup vs baseline: 1.2112x; 1.0328x over previous
"""Trainium2 Bass kernel for binarized 3x3 conv + batch-norm (BinConv2d).

Reference computation:
    xb = sign(x); wb = sign(weight)
    y  = conv2d(xb, wb, stride 1, pad 1)        # NCHW / OIHW
    out = batchnorm(y, batch stats over (N,H,W), affine gamma/beta)

Strategy: data-parallel over batch (64 images -> 8 images per NeuronCore).
The conv runs as shifted matmuls with Cin=128 on the SBUF partition dim,
accumulating in PSUM. Signs are cast to fp8 (e4m3, +/-1 exact) and the 3x3
taps are processed as 4 DoubleRow pairs + 1 single matmul per output tile
(~1.8x TensorE throughput vs bf16). Matmul tiles span 8 rows x 58 cols of
the zero-padded image so every tap's moving operand is one contiguous
464-element run; the two junk columns per row are skipped downstream.
Conv outputs are integers |y| <= 1152: exact in fp32 PSUM and in the fp16
SBUF copy. Channel stats come from DVE bn_stats/bn_aggr, are AllReduced
across the 8 cores, and the affine is applied on-device before the f32
output DMA.
"""
import numpy as np

import concourse.bacc as bacc
import concourse.tile as tile
import concourse.mybir as mybir
import concourse.bass_utils as bass_utils
from concourse.bass_types import AP

F32 = mybir.dt.float32
F16 = mybir.dt.float16
F8 = mybir.dt.float8e4
AF = mybir.ActivationFunctionType
ALU = mybir.AluOpType
DR = mybir.MatmulPerfMode.DoubleRow

N_CORES = 8
N_FULL = 64            # total batch
NIMG = N_FULL // N_CORES   # images per core
C = 128                # channels (in == out)
H = W = 56
WP = W + 2             # padded width (58)
HPHYS = H + 4          # physical rows: guard + pad + 56 + pad + guard
PSTRIDE = HPHYS * WP   # per-partition elements of one image tile
NT = 7                 # row tiles per image
RT = H // NT           # rows per tile (8)
TW = RT * WP           # moving free size per tile (464)
IMG = H * W            # 3136
COUNT = N_FULL * IMG   # global reduction count per channel
EPS = 1e-5

TRACE = False          # test.py may flip this to get an NTFF profile

_CACHE = {}


def _build(use_collective=True, nimg=NIMG):
    nc = bacc.Bacc("TRN2", target_bir_lowering=False, debug=False,
                   num_devices=N_CORES)
    x = nc.dram_tensor("x", [NIMG, C, H, W], F32, kind="ExternalInput").ap()
    wt = nc.dram_tensor("wt", [C, 9, C], F32, kind="ExternalInput").ap()
    gb = nc.dram_tensor("gb", [C, 2], F32, kind="ExternalInput").ap()
    out = nc.dram_tensor("out", [NIMG, C, H, W], F32, kind="ExternalOutput").ap()

    with tile.TileContext(nc) as tc:
        with tc.tile_pool(name="const", bufs=1) as pc, \
             tc.tile_pool(name="xstage", bufs=4) as pxs, \
             tc.tile_pool(name="xpad", bufs=3) as pxp, \
             tc.tile_pool(name="ostage", bufs=4) as pos, \
             tc.tile_pool(name="psum", bufs=8, space="PSUM") as pp, \
             tc.tile_pool(name="dram", bufs=1, space="DRAM") as pd:

            # ---- weights: load f32 [ci, tap, co], sign -> fp8 ----
            wstage = pc.tile([C, 9, C], F32)
            nc.sync.dma_start(out=wstage[:], in_=wt[:])
            wb = pc.tile([C, 9, C], F8)
            nc.scalar.activation(out=wb[:], in_=wstage[:], func=AF.Sign)

            gbt = pc.tile([C, 2], F32)
            nc.sync.dma_start(out=gbt[:], in_=gb[:])

            # ---- persistent buffers ----
            y16 = pc.tile([C, NIMG, H, W], F16)       # conv ints (exact)
            bnbuf = pc.tile([C, nimg * NT, 6], F32)
            stats = pc.tile([C, 2], F32)
            gstats = pc.tile([C, 2], F32)

            # warm up the collectives firmware early so the real AllReduce's
            # trigger latency overlaps the conv phase
            if use_collective:
                wbin = pd.tile([C, 1], F32)
                wbout = pd.tile([C, 1], F32)
                nc.sync.dma_start(out=wbin[:], in_=gbt[:, 0:1])
                nc.gpsimd.collective_compute(
                    "AllReduce", ALU.add,
                    replica_groups=[list(range(N_CORES))],
                    ins=[wbin.opt()], outs=[wbout.opt()])

            # ---- phase 1: conv + local stats, per image ----
            HH = H // 2
            for n in range(nimg):
                # physical rows: 0 guard, 1 top pad, 2..57 image, 58 bottom
                # pad, 59 guard. Guards keep the deliberate 2-junk-column
                # overreads of the 58-wide matmul tiles inside the tile.
                xp = pxp.tile([C, HPHYS, WP], F8)
                nc.gpsimd.memset(xp[:, 0:2, :], 0.0)
                nc.gpsimd.memset(xp[:, HPHYS - 2:HPHYS, :], 0.0)
                nc.gpsimd.memset(xp[:, 2:HPHYS - 2, 0], 0.0)
                nc.gpsimd.memset(xp[:, 2:HPHYS - 2, WP - 1], 0.0)
                # DMA + sign in half-image chunks so matmuls start sooner
                for h in (0, HH):
                    xs = pxs.tile([C, HH, W], F32, tag="xs", name="xs")
                    nc.sync.dma_start(out=xs[:], in_=x[n, :, h:h + HH, :])
                    nc.scalar.activation(
                        out=xp[:, 2 + h:2 + h + HH, 1:WP - 1],
                        in_=xs[:], func=AF.Sign)

                psums = [pp.tile([C, TW], F32, tag="ps", name="ps")
                         for _ in range(NT)]

                def tap_off(h0, it):
                    dh, dw = it // 3 - 1, it % 3 - 1
                    return (h0 + 2 + dh) * WP + dw

                # tap-step outer, tile inner: consecutive matmuls share the
                # stationary operand
                for p in range(5):
                    for t in range(NT):
                        h0 = t * RT
                        if p < 4:
                            o0 = tap_off(h0, 2 * p)
                            o1 = tap_off(h0, 2 * p + 1)
                            rhs = AP(xp.tensor, xp.offset + o0,
                                     [[PSTRIDE, C], [o1 - o0, 2], [1, TW]])
                            nc.tensor.matmul(out=psums[t][:],
                                             lhsT=wb[:, 2 * p:2 * p + 2, :],
                                             rhs=rhs, start=(p == 0),
                                             stop=False, perf_mode=DR)
                        else:
                            o8 = tap_off(h0, 8)
                            rhs8 = AP(xp.tensor, xp.offset + o8,
                                      [[PSTRIDE, C], [1, TW]])
                            nc.tensor.matmul(out=psums[t][:], lhsT=wb[:, 8, :],
                                             rhs=rhs8, start=False, stop=True)

                for t in range(NT):
                    idx = n * NT + t
                    ps3 = psums[t][:].rearrange("p (r c) -> p r c", r=RT)
                    ydst = y16[:, n, t * RT:(t + 1) * RT, :]
                    # PSUM -> fp16 copy of the valid columns, alternating
                    # engines to balance ACT vs DVE load
                    if t % 2 == 0:
                        nc.scalar.copy(out=ydst, in_=ps3[:, :, 1:W + 1])
                    else:
                        nc.vector.tensor_copy(out=ydst, in_=ps3[:, :, 1:W + 1])
                    # DVE: count/mean/M2 from the contiguous fp16 copy
                    nc.vector.bn_stats(
                        out=bnbuf[:, idx, :],
                        in_=ydst.rearrange("p r c -> p (r c)"))

            # ---- phase 2: global stats via AllReduce ----
            mv = pc.tile([C, 2], F32)
            nc.vector.bn_aggr(out=mv[:],
                              in_=bnbuf[:].rearrange("p a s -> p (a s)"))
            cnt = float(nimg * IMG)
            msq_t = pc.tile([C, 1], F32)
            nc.vector.tensor_mul(msq_t[:], mv[:, 0:1], mv[:, 0:1])
            nc.vector.tensor_scalar_mul(stats[:, 0:1], mv[:, 0:1], cnt)
            nc.vector.tensor_add(stats[:, 1:2], mv[:, 1:2], msq_t[:])
            nc.vector.tensor_scalar_mul(stats[:, 1:2], stats[:, 1:2], cnt)
            if use_collective:
                bin_ = pd.tile([C, 2], F32)
                bout = pd.tile([C, 2], F32)
                nc.sync.dma_start(out=bin_[:], in_=stats[:])
                nc.gpsimd.collective_compute(
                    "AllReduce", ALU.add,
                    replica_groups=[list(range(N_CORES))],
                    ins=[bin_.opt()], outs=[bout.opt()])
                nc.sync.dma_start(out=gstats[:], in_=bout[:])
            else:
                nc.vector.tensor_scalar_mul(gstats[:], stats[:],
                                            float(N_CORES))

            # scale = gamma / sqrt(var + eps); bias = beta - mean * scale
            mean_t = pc.tile([C, 1], F32)
            e2_t = pc.tile([C, 1], F32)
            var_t = pc.tile([C, 1], F32)
            std_t = pc.tile([C, 1], F32)
            inv_t = pc.tile([C, 1], F32)
            scale_t = pc.tile([C, 1], F32)
            bias_t = pc.tile([C, 1], F32)
            tmp_t = pc.tile([C, 1], F32)
            inv_count = 1.0 / COUNT
            nc.vector.tensor_scalar_mul(mean_t[:], gstats[:, 0:1], inv_count)
            nc.vector.tensor_scalar_mul(e2_t[:], gstats[:, 1:2], inv_count)
            nc.vector.tensor_mul(tmp_t[:], mean_t[:], mean_t[:])
            nc.vector.tensor_sub(var_t[:], e2_t[:], tmp_t[:])
            nc.vector.tensor_scalar_add(var_t[:], var_t[:], EPS)
            nc.scalar.activation(out=std_t[:], in_=var_t[:], func=AF.Sqrt)
            nc.vector.reciprocal(inv_t[:], std_t[:])
            nc.vector.tensor_mul(scale_t[:], gbt[:, 0:1], inv_t[:])
            nc.vector.tensor_mul(tmp_t[:], mean_t[:], scale_t[:])
            nc.vector.tensor_sub(bias_t[:], gbt[:, 1:2], tmp_t[:])

            # ---- phase 3: affine + store, half-image chunks on ACT+DVE ----
            for n in range(nimg):
                for ci, h in enumerate((0, HH)):
                    ot = pos.tile([C, HH, W], F32, tag="ot", name="ot")
                    ysrc = y16[:, n, h:h + HH, :]
                    if (2 * n + ci) % 2 == 0:
                        nc.vector.tensor_scalar(
                            ot[:], ysrc, scale_t[:, 0:1], bias_t[:, 0:1],
                            ALU.mult, ALU.add)
                    else:
                        nc.scalar.activation(
                            out=ot[:], in_=ysrc, func=AF.Identity,
                            bias=bias_t[:, 0:1], scale=scale_t[:, 0:1])
                    nc.sync.dma_start(out=out[n, :, h:h + HH, :], in_=ot[:])

    nc.compile()
    return nc


def kernel(x, weight, gamma, beta):
    x = np.asarray(x, dtype=np.float32)
    weight = np.asarray(weight, dtype=np.float32)
    gamma = np.asarray(gamma, dtype=np.float32)
    beta = np.asarray(beta, dtype=np.float32)

    if "nc" not in _CACHE:
        _CACHE["nc"] = _build()
    nc = _CACHE["nc"]

    # wt[ci, kh*3+kw, co] = weight[co, ci, kh, kw]
    wt = np.ascontiguousarray(weight.transpose(1, 2, 3, 0)).reshape(C, 9, C)
    gb = np.ascontiguousarray(np.stack([gamma, beta], axis=1))

    in_maps = []
    for i in range(N_CORES):
        in_maps.append({
            "x": np.ascontiguousarray(x[i * NIMG:(i + 1) * NIMG]),
            "wt": wt,
            "gb": gb,
        })

    res = bass_utils.run_bass_kernel_spmd(
        nc, in_maps, core_ids=list(range(N_CORES)), trace=TRACE)
    _CACHE["last_result"] = res

    out = np.empty((N_FULL, C, H, W), dtype=np.float32)
    for i in range(N_CORES):
        out[i * NIMG:(i + 1) * NIMG] = res.results[i]["out"]
    return out
